# revision 1
# baseline (speedup 1.0000x reference)
"""Bidirectional masked LSTM encoder (B=512, T=1024, EMB=HID=64) on 8 TRN2 cores.

Strategy: data-parallel over batch (64 samples/core). Per core, the forward and
backward LSTM streams run as two independent instruction streams that the Tile
scheduler staggers to hide per-step latency.

Per direction/step:
  zq[128,128] (PSUM) = [i;f | o;2g] gates via 4 matmuls:
     x-part: K=65 (emb row + mask-indicator row as bias multiplier), start=True
     h-part: K=64, accumulate
  S = sigmoid(zq)                       (one ACT op; tanh(g)=2*sigmoid(2g)-1)
  u = f*c (DVE); v = i*gp (DVE); t2 = 2v+u (gpsimd); cn = t2-i (gpsimd)
  tc = tanh(cn) (ACT); p = o*tc (gpsimd)
  state[c|h] <- copy_predicated(mask, [cn|p])   (DVE; Keras mask_zero carry)

Embedding gather: gpsimd ap_gather from SBUF-resident table [128, 1000]
(rows 0:64 emb.T, rows 64:128 = (token!=0) indicator), 64-step chunks.
"""

import numpy as np

VOCAB = 1000
EMB = 64
HID = 64
B_FULL = 512
T_FULL = 1024
N_CORES = 8
B = B_FULL // N_CORES  # 64 per core

_COMPILED = {}


# ----------------------------------------------------------------------------
# Host-side input packing
# ----------------------------------------------------------------------------

def _pack_wrapped_idx(jw: np.ndarray) -> np.ndarray:
    """Pack a flat index stream into ap_gather's wrapped layout.

    Index j lives at partition j%16, free slot j//16, replicated into each of
    the eight 16-partition blocks (one per gpsimd core).
    """
    n = jw.shape[0]
    assert n % 16 == 0
    w = jw.reshape(n // 16, 16).T.astype(np.int16)  # [16, n/16]
    return np.tile(w, (8, 1))  # [128, n/16]


def _host_prep_shared(emb_table, Wx_f, Wh_f, b_f, Wx_b, Wh_b, b_b):
    """Weight/table tensors shared by all cores."""
    f32 = np.float32

    def packs(Wx, Wh, b):
        # gate order in reference: z -> i, f, g, o (cols 0:64,64:128,128:192,192:256)
        # pair "if" = (i, f) cols 0:128; pair "og" = (o, 2*g).
        lx_if = np.vstack([Wx[:, 0:128], b[None, 0:128]]).astype(f32)          # [65,128]
        og_w = np.hstack([2.0 * Wx[:, 128:192], Wx[:, 192:256]])
        og_b = np.concatenate([2.0 * b[128:192], b[192:256]])
        lx_og = np.vstack([og_w, og_b[None, :]]).astype(f32)                   # [65,128]
        lh_if = Wh[:, 0:128].astype(f32)                                       # [64,128]
        lh_og = np.hstack([2.0 * Wh[:, 128:192], Wh[:, 192:256]]).astype(f32)  # [64,128]
        return lx_if, lx_og, lh_if, lh_og

    lx_if_f, lx_og_f, lh_if_f, lh_og_f = packs(Wx_f, Wh_f, b_f)
    lx_if_b, lx_og_b, lh_if_b, lh_og_b = packs(Wx_b, Wh_b, b_b)

    ind = (np.arange(VOCAB) != 0).astype(f32)
    tab = np.vstack([emb_table.T.astype(f32), np.tile(ind[None, :], (64, 1))])  # [128,1000]

    return {
        "tab": tab,
        "lx_if_f": lx_if_f, "lx_og_f": lx_og_f, "lx_if_b": lx_if_b, "lx_og_b": lx_og_b,
        "lh_if_f": lh_if_f, "lh_og_f": lh_og_f, "lh_if_b": lh_if_b, "lh_og_b": lh_og_b,
    }


def _host_prep_core(tok_c: np.ndarray, T: int) -> np.ndarray:
    """Per-core gather-index stream: fwd (t ascending) then bwd (t descending)."""
    jw_f = tok_c.T.reshape(-1)                # j = t*B + b
    jw_b = tok_c[:, ::-1].T.reshape(-1)
    return np.concatenate([_pack_wrapped_idx(jw_f), _pack_wrapped_idx(jw_b)], axis=1)


# ----------------------------------------------------------------------------
# Device program
# ----------------------------------------------------------------------------

def _build_body(tc, outs, ins, T: int, knobs=None):
    import concourse.bass as bass
    from concourse import mybir

    f32 = mybir.dt.float32
    Sig = mybir.ActivationFunctionType.Sigmoid
    Tanh = mybir.ActivationFunctionType.Tanh
    Op = mybir.AluOpType

    from contextlib import ExitStack

    nc = tc.nc
    TC = 64                      # steps per gather chunk
    NCH = T // TC                # chunks per stream
    out = outs["out"]

    stack = ExitStack()
    def pool(name, bufs, **kw):
        return stack.enter_context(tc.tile_pool(name=name, bufs=bufs, **kw))

    kn = {"gbuf": 4, "zq": 6, "sg": 4, "work": 3}
    kn.update(knobs or {})
    consts = pool("consts", 1)
    gpool = pool("gbuf", kn["gbuf"])
    zqpool = pool("zq", kn["zq"], space="PSUM")
    spool = pool("sg", kn["sg"])
    work = pool("work", kn["work"])
    stpool = pool("state", 1)

    # --- constants into SBUF
    tab = consts.tile([128, VOCAB], f32)
    nc.sync.dma_start(out=tab, in_=ins["tab"])
    idx = consts.tile([128, 2 * T * B // 16], mybir.dt.int16)
    nc.sync.dma_start(out=idx, in_=ins["idx16"])

    W = {}
    for d in ("f", "b"):
        for p_ in ("if", "og"):
            wx = consts.tile([65, 128], f32, tag=f"lx_{p_}_{d}")
            nc.sync.dma_start(out=wx, in_=ins[f"lx_{p_}_{d}"])
            wh_t = consts.tile([128, 128], f32, tag=f"lh_{p_}_{d}")
            nc.sync.dma_start(out=wh_t[64:128, :], in_=ins[f"lh_{p_}_{d}"])
            W[f"x_{p_}_{d}"] = wx
            W[f"h_{p_}_{d}"] = wh_t[64:128, :]

    # --- per-stream persistent state: [64, 132] = (c @ 0:64 | h @ 68:132).
    # The 68-stride keeps the (c|h) pair non-contiguous so the combined
    # predicated state update stays a 3D AP after scheduler canonicalization.
    HB = 68
    state = {}
    for s in range(2):
        st_t = stpool.tile([128, 2 * HB], f32, tag=f"state{s}")
        nc.vector.memset(st_t, 0.0)
        state[s] = st_t[64:128, :]

    def two_block(ap2):
        return bass.AP(tensor=ap2.tensor, offset=ap2.offset,
                       ap=[ap2.ap[0], [HB, 2], [1, 64]])

    tab3 = tab.rearrange("c (n d) -> c n d", d=1)

    gbufs = {0: {}, 1: {}}

    def issue_gather(s, c):
        g = gpool.tile([128, TC * B], f32, tag="gbuf")
        g3 = g.rearrange("c (n d) -> c n d", d=1)
        nc.gpsimd.ap_gather(
            g3, tab3, idx[:, (s * T + c * TC) * B // 16:(s * T + (c + 1) * TC) * B // 16],
            channels=128, num_elems=VOCAB, d=1, num_idxs=TC * B,
        )
        gbufs[s][c] = g

    # prime first chunks of both streams
    issue_gather(0, 0)
    issue_gather(1, 0)

    dnames = ("f", "b")
    for n in range(T):
        c = n // TC
        if n % TC == 0 and c + 1 < NCH:
            issue_gather(0, c + 1)
            issue_gather(1, c + 1)
        for s in (0, 1):
            d = dnames[s]
            st = state[s]
            g = gbufs[s][c]
            col = (n % TC) * B
            gx = g[0:65, col:col + B]            # [65, B] x rows + indicator row
            zq = zqpool.tile([128, 128], f32, tag="zq")
            nc.tensor.matmul(zq[:, 0:64], W[f"x_if_{d}"], gx, start=True, stop=False)
            nc.tensor.matmul(zq[:, 0:64], W[f"h_if_{d}"], st[:, HB:HB + 64], start=False, stop=True)
            nc.tensor.matmul(zq[:, 64:128], W[f"x_og_{d}"], gx, start=True, stop=False)
            nc.tensor.matmul(zq[:, 64:128], W[f"h_og_{d}"], st[:, HB:HB + 64], start=False, stop=True)

            S = spool.tile([128, 128], f32, tag="S")
            nc.scalar.activation(S, zq, Sig)
            # i=S[0:64,0:64] f=S[64:128,0:64] gp=S[0:64,64:128] o=S[64:128,64:128]
            # cn = f*c + 2*i*gp - i ; engines chosen to satisfy base-partition
            # pairing (2-input SBUF ops need equal input bases; gpsimd ops
            # additionally keep out at the same base).
            u_t = work.tile([128, 64], f32, tag="u")
            nc.gpsimd.tensor_tensor(u_t[64:128, :], S[64:128, 0:64], st[:, 0:64], op=Op.mult)
            v = work.tile([64, 64], f32, tag="v")
            nc.vector.tensor_tensor(v, S[0:64, 0:64], S[0:64, 64:128], op=Op.mult)
            t3_t = work.tile([128, 64], f32, tag="t3")
            nc.vector.scalar_tensor_tensor(t3_t[64:128, :], v, 2.0, S[0:64, 0:64], op0=Op.mult, op1=Op.subtract)
            new2_t = work.tile([128, 2 * HB], f32, tag="new2")
            new2 = new2_t[64:128, :]
            nc.gpsimd.tensor_tensor(new2[:, 0:64], t3_t[64:128, :], u_t[64:128, :], op=Op.add)
            tc_t = work.tile([128, 64], f32, tag="tcn")
            nc.scalar.activation(tc_t[64:128, :], new2[:, 0:64], Tanh)
            nc.gpsimd.tensor_tensor(new2[:, HB:HB + 64], S[64:128, 64:128], tc_t[64:128, :], op=Op.mult)
            mask2 = g[64:128, col:col + B].bitcast(mybir.dt.uint32).unsqueeze(1).broadcast_to([64, 2, B])
            nc.vector.copy_predicated(two_block(st), mask2, two_block(new2))

    # --- write out: out[b, 0:64] = h_f[:, b]; out[b, 64:128] = h_b[:, b]
    for s in range(2):
        h = state[s][:, HB:HB + 64]
        dst = out[:, s * HID:(s + 1) * HID].transpose((1, 0))  # [hid, b] view of dram
        nc.sync.dma_start(out=dst, in_=h)

    stack.close()


def _compile(T: int, knobs=None):
    import concourse.bacc as bacc
    import concourse.tile as tile
    from concourse import mybir

    key = (T, tuple(sorted((knobs or {}).items())))
    if key in _COMPILED:
        return _COMPILED[key]

    nc = bacc.Bacc("TRN2", num_devices=N_CORES)
    f32 = mybir.dt.float32
    i16 = mybir.dt.int16

    ins = {}
    def din(name, shape, dtype):
        ins[name] = nc.dram_tensor(name, shape, dtype, kind="ExternalInput").ap()

    din("tab", [128, VOCAB], f32)
    din("idx16", [128, 2 * T * B // 16], i16)
    for d in ("f", "b"):
        din(f"lx_if_{d}", [65, 128], f32)
        din(f"lx_og_{d}", [65, 128], f32)
        din(f"lh_if_{d}", [64, 128], f32)
        din(f"lh_og_{d}", [64, 128], f32)
    out = nc.dram_tensor("out", [B, 2 * HID], f32, kind="ExternalOutput").ap()

    with tile.TileContext(nc) as tc:
        _build_body(tc, {"out": out}, ins, T=T, knobs=knobs)
    nc.compile()

    _COMPILED[key] = (nc, list(ins.keys()))
    return _COMPILED[key]


def kernel(tokens, emb_table, Wx_f, Wh_f, b_f, Wx_b, Wh_b, b_b):
    from concourse import bass_utils

    tokens = np.asarray(tokens)
    T = tokens.shape[1]
    nc, in_names = _compile(T)

    shared = _host_prep_shared(
        np.asarray(emb_table), np.asarray(Wx_f), np.asarray(Wh_f), np.asarray(b_f),
        np.asarray(Wx_b), np.asarray(Wh_b), np.asarray(b_b))

    in_maps = []
    for c in range(N_CORES):
        tok_c = tokens[c * B:(c + 1) * B]
        m = dict(shared)
        m["idx16"] = _host_prep_core(tok_c, T)
        in_maps.append(m)

    res = bass_utils.run_bass_kernel_spmd(nc, in_maps, core_ids=list(range(N_CORES)))
    global _LAST_RESULTS, _LAST_EXEC_NS
    _LAST_RESULTS = res
    _LAST_EXEC_NS = getattr(res, "exec_time_ns", None)
    outs = [res.results[c]["out"] for c in range(N_CORES)]
    return np.concatenate(outs, axis=0).astype(np.float32)

